# revision 45
# baseline (speedup 1.0000x reference)
"""Distributed causal-attention kernel for TRN2 (8 NeuronCores).

Module: qkv = x@w_attn+b; q,k l2-normalized per head; scaled (8.0) causal
softmax attention; out = (attn@v reassembled)@w_proj + b_proj.
Shapes: x [2,2048,1024], 16 heads x 64 dim.

Sharding: pure tensor-parallel over heads (2 heads/core).  Each core
computes qkv for its heads over the full batch*seq, runs attention, then two
8-core AllToAlls (one per head, pipelined against compute) redistribute the
per-head outputs to row-shards so each core applies the full output
projection to its 512 rows.

Key device-side choices:
 - host passes x transposed; qkv lands in [seq, cols] layout where q,k are
   normalized with free-axis norms (all per-tile norm stats batched so ACT
   loads the Ln/Exp table sets once instead of thrashing per tile), then
   PE-transposed to [hd, seq]
 - scores are computed transposed [k, q]; the exp'd tile is directly the
   AV matmul's stationary operand; the softmax denominator comes from a
   ones column appended to v; the divide is a per-partition scalar multiply
   on the [q, hd] AV output
 - each q-subtile accumulates in its own PSUM bank (matmul start=True
   clears has_written for its whole 2KB zero region, so concurrent
   accumulation groups must never share a bank)
 - o is transposed on-device pre-A2A so the collective payload is oT and
   the receive side is a single plain DMA into the projection layout
 - the output projection runs in two K=64 passes (one per head) so the
   first half overlaps the second collective
"""
import sys

if '/opt/trn_rl_repo' not in sys.path:
    sys.path.insert(0, '/opt/trn_rl_repo')

import numpy as np
import ml_dtypes

import concourse.bass as bass
import concourse.mybir as mybir
from concourse import bacc, tile
from concourse.bass import ts, ds
from concourse.bass_utils import run_bass_kernel_spmd
from concourse.masks import make_identity

B, S, D, H = 2, 2048, 1024, 16
HD = D // H                 # 64
NCORES = 8
HPC = H // NCORES           # 2 heads per core
SEQT = 128
NT = (B * S) // SEQT        # 32 seq tiles (batch-major)
TPB = S // SEQT             # 16 tiles per batch
QSPAN = 512
NSPAN = S // QSPAN          # 4 q-spans per batch
ROWS = (B * S) // NCORES    # 512 output rows per core
KC = D // 128               # 8 contraction chunks
W3 = 3 * HPC * HD           # 384 qkv columns per core
BF = mybir.dt.bfloat16
F32 = mybir.dt.float32
HALF_LN8 = 1.0397207708399179  # 0.5*ln(8): folds the 8.0 score scale
AF = mybir.ActivationFunctionType
MUL = mybir.AluOpType.mult


def build(dbg=False, with_bias=False):
    nc = bacc.Bacc("TRN2", target_bir_lowering=False, debug=False,
                   num_devices=NCORES)
    xt = nc.dram_tensor("xt", [D, B * S], BF, kind="ExternalInput")
    wq = nc.dram_tensor("wq", [D, W3], BF, kind="ExternalInput")
    ba = nc.dram_tensor("ba", [1, W3], BF, kind="ExternalInput")
    wp = nc.dram_tensor("wp", [D, D], BF, kind="ExternalInput")
    bp = nc.dram_tensor("bp", [1, D], BF, kind="ExternalInput")
    out = nc.dram_tensor("out", [ROWS, D], F32, kind="ExternalOutput")
    if dbg:
        d_qt = nc.dram_tensor("d_qt", [128, S], BF, kind="ExternalOutput")
        d_kt = nc.dram_tensor("d_kt", [128, S], BF, kind="ExternalOutput")
        d_v = nc.dram_tensor("d_v", [128, NT * 2 * (HD + 1)], BF,
                             kind="ExternalOutput")
        d_oc = nc.dram_tensor("d_oc", [128, NCORES * ROWS], BF,
                              kind="ExternalOutput")

    with tile.TileContext(nc) as tc:
        with tc.tile_pool(name="persist", bufs=1) as pp, \
             tc.tile_pool(name="dram", bufs=1, space="DRAM") as dram, \
             tc.tile_pool(name="work", bufs=4) as work, \
             tc.tile_pool(name="epool", bufs=12) as epool:

            # ---- persistent SBUF ----
            xt_sb = pp.tile([128, KC, B * S], BF, name="xt_sb")
            wq_sb = pp.tile([128, KC, W3], BF, name="wq_sb")
            wp_sb = [pp.tile([64, KC, D], BF, name=f"wp_sb{h}")
                     for h in range(HPC)]
            ba_sb = pp.tile([1, W3], BF, name="ba_sb")
            bp_sb = pp.tile([1, D], BF, name="bp_sb")
            ones_sb = pp.tile([1, 128], BF, name="ones_sb")
            c_bias = pp.tile([128, 1], F32, name="c_bias")
            c_scale = pp.tile([128, 1], F32, name="c_scale")
            ident = pp.tile([128, 128], BF, name="ident")
            tri = pp.tile([128, 128], BF, name="tri")
            # q,k working copies (normalized in place) + batched norm stats
            qk_all = pp.tile([128, NT, 2 * HPC * HD], BF, name="qk_all")
            n2_all = pp.tile([128, NT, 2 * HPC], F32, name="n2_all")
            rn_all = pp.tile([128, NT, 2 * HPC], F32, name="rn_all")
            # qT/kT per batch: head0 rows 0:64, head1 rows 64:128
            QT = [pp.tile([128, S], BF, name=f"qt{b}") for b in range(B)]
            KT = [pp.tile([128, S], BF, name=f"kt{b}") for b in range(B)]
            # v in [seq, hd] layout, per head augmented with a ones column
            v_sb = pp.tile([128, NT, 2 * (HD + 1)], BF, name="v_sb")
            ocT = [pp.tile([64, NCORES, ROWS], BF, name=f"ocT{h}")
                   for h in range(HPC)]

            # per-head A2A payload is oT: shard p = rows [64p:64p+64] = this
            # core's head-h oT columns destined for core p
            a2a_in = [dram.tile([64 * NCORES, ROWS], BF, name=f"a2a_in{h}")
                      for h in range(HPC)]
            a2a_out = [dram.tile([64 * NCORES, ROWS], BF, name=f"a2a_out{h}")
                       for h in range(HPC)]

            # ---- constants (ordered so phase B can start early) ----
            for kc in range(KC):
                nc.sync.dma_start(wq_sb[:, kc, :], wq[ts(kc, 128), :])
            nc.sync.dma_start(ba_sb[:], ba[:])
            nc.sync.dma_start(bp_sb[:], bp[:])
            for tq in range(4):
                for kc in range(KC):
                    nc.sync.dma_start(xt_sb[:, kc, ds(1024 * tq, 1024)],
                                      xt[ts(kc, 128), ds(1024 * tq, 1024)])
            for kc in range(KC):
                for h in range(HPC):
                    nc.sync.dma_start(wp_sb[h][:, kc, :],
                                      wp[128 * kc + 64 * h:
                                         128 * kc + 64 * h + 64, :])
            nc.gpsimd.memset(ones_sb[:], 1.0)
            nc.gpsimd.memset(c_bias[:], HALF_LN8)
            nc.gpsimd.memset(c_scale[:], -0.5)
            nc.gpsimd.memset(v_sb[:], 1.0)
            make_identity(nc, ident[:])
            # tri[k, q] = 1 where q >= k (valid causal), else 0
            nc.gpsimd.memset(tri[:], 1.0)
            nc.gpsimd.affine_select(
                out=tri[:], in_=tri[:], compare_op=mybir.AluOpType.is_ge,
                fill=0.0, base=0, pattern=[[1, 128]], channel_multiplier=-1)

            # ---- phase B1: qkv matmuls + norm stats ----
            ps_qkv_ctx = tc.tile_pool(name="ps_qkv", bufs=4, space="PSUM")
            ps_qkv = ps_qkv_ctx.__enter__()
            for t in range(NT):
                ps = ps_qkv.tile([128, W3], F32, tag="ps", name=f"ps{t}")
                for kc in range(KC):
                    nc.tensor.matmul(ps[:], lhsT=xt_sb[:, kc, ts(t, 128)],
                                     rhs=wq_sb[:, kc, :], start=(kc == 0),
                                     stop=(not with_bias and kc == KC - 1))
                if with_bias:
                    nc.tensor.matmul(ps[:], lhsT=ones_sb[:, 0:128],
                                     rhs=ba_sb[:], start=False, stop=True)
                nc.scalar.copy(qk_all[:, t, :], ps[:, 0:256])
                nc.vector.tensor_copy(
                    v_sb[:, t, :].rearrange(
                        "p (h e) -> p h e", e=HD + 1)[:, :, 0:HD],
                    ps[:, 256:384].rearrange("p (h e) -> p h e", e=HD))
                sq = work.tile([128, 2 * HPC * HD], BF, tag="sq",
                               name=f"sq{t}")
                nc.vector.tensor_mul(sq[:], qk_all[:, t, :], qk_all[:, t, :])
                nc.vector.reduce_sum(
                    n2_all[:, t, :], sq[:].rearrange("p (g e) -> p g e", e=HD),
                    axis=mybir.AxisListType.X)

            # batched rnorm = sqrt(8)/sqrt(n2): one Ln + one Exp (one table
            # set load each instead of per-tile thrash)
            nc.scalar.activation(rn_all[:].rearrange("p a b -> p (a b)"),
                                 n2_all[:].rearrange("p a b -> p (a b)"), AF.Ln)
            nc.scalar.activation(rn_all[:].rearrange("p a b -> p (a b)"),
                                 rn_all[:].rearrange("p a b -> p (a b)"), AF.Exp,
                                 scale=c_scale[:], bias=c_bias[:])

            ps_qkv_ctx.__exit__(None, None, None)
            ps_tr_ctx = tc.tile_pool(name="ps_tr", bufs=1, space="PSUM")
            ps_tr = ps_tr_ctx.__enter__()
            psB_ctx = tc.tile_pool(name="psB", bufs=3, space="PSUM")
            psB = psB_ctx.__enter__()
            psC_ctx = tc.tile_pool(name="psC", bufs=4, space="PSUM")
            psC = psC_ctx.__enter__()

            # ---- phase B2: normalize + transpose ----
            for t in range(NT):
                b_, tt = divmod(t, TPB)
                nc.vector.tensor_tensor(
                    qk_all[:, t, :].rearrange("p (g e) -> p g e", e=HD),
                    qk_all[:, t, :].rearrange("p (g e) -> p g e", e=HD),
                    rn_all[:, t, :, None].broadcast_to([128, 4, HD]), op=MUL)
                for src0, dst in ((0, QT[b_]), (128, KT[b_])):
                    trp = ps_tr.tile([128, 128], BF, tag="tr",
                                     name=f"tr{t}_{src0}")
                    nc.tensor.transpose(
                        trp[:], qk_all[:, t, src0:src0 + 128], ident[:])
                    nc.vector.tensor_copy(dst[:, ts(tt, 128)], trp[:])

            # ---- phase C: attention (head-major for A2A pipelining) ----
            for h in range(HPC):
                for b_ in range(B):
                    for j in range(NSPAN):
                        oaccs = [work.tile([128, HD], BF, tag="oacc", bufs=8,
                                           name=f"oacc{h}_{b_}_{j}_{c}")
                                 for c in range(4)]
                        nk = 4 * j + 4
                        avs = [psC.tile([128, HD + 1], F32, tag="av",
                                        name=f"av{b_}_{j}_{h}_{c}")
                               for c in range(4)]
                        for i in range(nk):
                            d = i - 4 * j
                            c0 = max(d, 0)
                            sps = psB.tile([128, QSPAN], F32, tag="s",
                                           name=f"s{b_}_{j}_{h}_{i}")
                            # stream only the causally-needed q columns
                            nc.tensor.matmul(
                                sps[:, 128 * c0:],
                                lhsT=KT[b_][64 * h:64 * h + 64, ts(i, 128)],
                                rhs=QT[b_][64 * h:64 * h + 64,
                                           ds(j * QSPAN + 128 * c0,
                                              QSPAN - 128 * c0)],
                                start=True, stop=True)
                            e = epool.tile([128, QSPAN], BF, tag="e",
                                           name=f"e{b_}_{j}_{h}_{i}")
                            nc.scalar.activation(e[:, 128 * c0:],
                                                 sps[:, 128 * c0:], AF.Exp)
                            if d >= 0:
                                nc.vector.tensor_tensor(
                                    e[:, 128 * d:128 * (d + 1)],
                                    e[:, 128 * d:128 * (d + 1)], tri[:],
                                    op=MUL)
                            for c in range(c0, 4):
                                nc.tensor.matmul(
                                    avs[c][:],
                                    lhsT=e[:, ts(c, 128)],
                                    rhs=v_sb[:, b_ * TPB + i,
                                             65 * h:65 * h + 65],
                                    start=(i == 0), stop=(i == 4 * j + c))
                        rd = work.tile([128, 4], F32, tag="rd",
                                       name=f"rd{b_}_{j}_{h}")
                        for c in range(4):
                            nc.vector.reciprocal(rd[:, c:c + 1],
                                                 avs[c][:, HD:HD + 1])
                            oacc = oaccs[c]
                            nc.vector.tensor_scalar_mul(
                                oacc[:], avs[c][:, 0:HD], rd[:, c:c + 1])
                            g = b_ * TPB + 4 * j + c
                            trp2 = ps_tr.tile([64, 128], BF, tag="tr",
                                              name=f"ot{h}_{g}")
                            nc.tensor.transpose(trp2[:], oacc[:], ident[:])
                            ot = work.tile([64, 128], BF, tag="ot", bufs=6,
                                           name=f"ots{h}_{g}")
                            nc.vector.tensor_copy(ot[:], trp2[:])
                            nc.sync.dma_start(
                                a2a_in[h][ts(g // 4, 64),
                                          ds((g % 4) * 128, 128)],
                                ot[:])
                # per-head A2A fires as soon as this head's o is out;
                # head 0's collective overlaps head 1's attention
                nc.gpsimd.collective_compute(
                    "AllToAll", mybir.AluOpType.bypass,
                    replica_groups=[list(range(NCORES))],
                    ins=[a2a_in[h][:].opt()], outs=[a2a_out[h][:].opt()])
                nc.sync.dma_start(
                    ocT[h][:, :, :],
                    a2a_out[h][:].rearrange("(p c) w -> c p w", c=64))

            if dbg:
                nc.sync.dma_start(d_qt[:], QT[0][:])
                nc.sync.dma_start(d_kt[:], KT[0][:])
                nc.sync.dma_start(d_v[:],
                                  v_sb[:].rearrange("p a b -> p (a b)"))
                for h in range(HPC):
                    nc.sync.dma_start(
                        d_oc[64 * h:64 * h + 64, :],
                        ocT[h][:].rearrange("p a b -> p (a b)"))

            # ---- phase D: projection, split per head so the h0 half
            # overlaps head 1's attention + A2A ----
            ysbs = {}
            for rt in range(ROWS // 128):
                for half in range(2):
                    yps = psB.tile([128, 512], F32, tag="s",
                                   name=f"y0_{rt}_{half}")
                    for p in range(NCORES):
                        nc.tensor.matmul(
                            yps[:], lhsT=ocT[0][:, p, ts(rt, 128)],
                            rhs=wp_sb[0][:, p, ds(half * 512, 512)],
                            start=(p == 0),
                            stop=(not with_bias and p == NCORES - 1))
                    if with_bias:
                        nc.tensor.matmul(yps[:], lhsT=ones_sb[:, 0:128],
                                         rhs=bp_sb[:, ds(half * 512, 512)],
                                         start=False, stop=True)
                    ysb = work.tile([128, 512], F32, tag="y", bufs=8,
                                    name=f"ysb{rt}_{half}")
                    nc.vector.tensor_copy(ysb[:], yps[:])
                    ysbs[rt, half] = ysb
            for rt in range(ROWS // 128):
                for half in range(2):
                    yps = psB.tile([128, 512], F32, tag="s",
                                   name=f"y1_{rt}_{half}")
                    for p in range(NCORES):
                        nc.tensor.matmul(
                            yps[:], lhsT=ocT[1][:, p, ts(rt, 128)],
                            rhs=wp_sb[1][:, p, ds(half * 512, 512)],
                            start=(p == 0), stop=(p == NCORES - 1))
                    ysb = ysbs[rt, half]
                    nc.vector.tensor_tensor(ysb[:], ysb[:], yps[:],
                                            op=mybir.AluOpType.add)
                    nc.sync.dma_start(
                        out[ts(rt, 128), ds(half * 512, 512)], ysb[:])

            psC_ctx.__exit__(None, None, None)
            psB_ctx.__exit__(None, None, None)
            ps_tr_ctx.__exit__(None, None, None)

    nc.compile()
    return nc


_NC = None


def _get_nc(with_bias=False):
    global _NC
    if _NC is None or _NC[1] != with_bias:
        _NC = (build(with_bias=with_bias), with_bias)
    return _NC[0]


def make_in_maps(x, w_attn, b_attn, w_proj, b_proj):
    bf = ml_dtypes.bfloat16
    xt = np.ascontiguousarray(x.reshape(B * S, D).T).astype(bf)
    wp_ = np.ascontiguousarray(w_proj).astype(bf)
    bp_ = b_proj.reshape(1, D).astype(bf)
    in_maps = []
    for c in range(NCORES):
        sl = slice(128 * c, 128 * c + 128)
        wq_ = np.ascontiguousarray(np.concatenate(
            [w_attn[:, sl], w_attn[:, 1024:2048][:, sl],
             w_attn[:, 2048:3072][:, sl]], axis=1)).astype(bf)
        ba_ = np.concatenate(
            [b_attn[sl], b_attn[1024:2048][sl],
             b_attn[2048:3072][sl]]).reshape(1, W3).astype(bf)
        in_maps.append({"xt": xt, "wq": wq_, "ba": ba_, "wp": wp_, "bp": bp_})
    return in_maps


def gather_out(results):
    out = np.empty((B, S, D), np.float32)
    for c in range(NCORES):
        out[c // 4, ROWS * (c % 4):ROWS * (c % 4 + 1), :] = results[c]["out"]
    return out


def kernel(x, w_attn, b_attn, w_proj, b_proj):
    with_bias = bool(np.any(b_attn) or np.any(b_proj))
    nc = _get_nc(with_bias=with_bias)
    in_maps = make_in_maps(np.asarray(x, np.float32), np.asarray(w_attn, np.float32),
                           np.asarray(b_attn, np.float32),
                           np.asarray(w_proj, np.float32),
                           np.asarray(b_proj, np.float32))
    res = run_bass_kernel_spmd(nc, in_maps, core_ids=list(range(NCORES)))
    return gather_out(res.results)


# revision 46
# speedup vs baseline: 1.0424x; 1.0424x over previous
"""Distributed causal-attention kernel for TRN2 (8 NeuronCores).

Module: qkv = x@w_attn+b; q,k l2-normalized per head; scaled (8.0) causal
softmax attention; out = (attn@v reassembled)@w_proj + b_proj.
Shapes: x [2,2048,1024], 16 heads x 64 dim.

Sharding: pure tensor-parallel over heads (2 heads/core).  Each core
computes qkv for its heads over the full batch*seq, runs attention, then two
8-core AllToAlls (one per head, pipelined against compute) redistribute the
per-head outputs to row-shards so each core applies the full output
projection to its 512 rows.

Key device-side choices:
 - host passes x transposed; qkv lands in [seq, cols] layout where q,k are
   normalized with free-axis norms (all per-tile norm stats batched so ACT
   loads the Ln/Exp table sets once instead of thrashing per tile), then
   PE-transposed to [hd, seq]
 - scores are computed transposed [k, q]; the exp'd tile is directly the
   AV matmul's stationary operand; the softmax denominator comes from a
   ones column appended to v; the divide is a per-partition scalar multiply
   on the [q, hd] AV output
 - each q-subtile accumulates in its own PSUM bank (matmul start=True
   clears has_written for its whole 2KB zero region, so concurrent
   accumulation groups must never share a bank)
 - o is transposed on-device pre-A2A so the collective payload is oT and
   the receive side is a single plain DMA into the projection layout
 - the output projection runs in two K=64 passes (one per head) so the
   first half overlaps the second collective
"""
import sys

if '/opt/trn_rl_repo' not in sys.path:
    sys.path.insert(0, '/opt/trn_rl_repo')

import numpy as np
import ml_dtypes

import concourse.bass as bass
import concourse.mybir as mybir
from concourse import bacc, tile
from concourse.bass import ts, ds
from concourse.bass_utils import run_bass_kernel_spmd
from concourse.masks import make_identity

B, S, D, H = 2, 2048, 1024, 16
HD = D // H                 # 64
NCORES = 8
HPC = H // NCORES           # 2 heads per core
SEQT = 128
NT = (B * S) // SEQT        # 32 seq tiles (batch-major)
TPB = S // SEQT             # 16 tiles per batch
QSPAN = 512
NSPAN = S // QSPAN          # 4 q-spans per batch
ROWS = (B * S) // NCORES    # 512 output rows per core
KC = D // 128               # 8 contraction chunks
W3 = 3 * HPC * HD           # 384 qkv columns per core
BF = mybir.dt.bfloat16
F32 = mybir.dt.float32
HALF_LN8 = 1.0397207708399179  # 0.5*ln(8): folds the 8.0 score scale
AF = mybir.ActivationFunctionType
MUL = mybir.AluOpType.mult


def build(dbg=False, with_bias=False):
    nc = bacc.Bacc("TRN2", target_bir_lowering=False, debug=False,
                   num_devices=NCORES)
    xt = nc.dram_tensor("xt", [D, B * S], BF, kind="ExternalInput")
    wq = nc.dram_tensor("wq", [D, W3], BF, kind="ExternalInput")
    ba = nc.dram_tensor("ba", [1, W3], BF, kind="ExternalInput")
    wp = nc.dram_tensor("wp", [D, D], BF, kind="ExternalInput")
    bp = nc.dram_tensor("bp", [1, D], BF, kind="ExternalInput")
    out = nc.dram_tensor("out", [ROWS, D], F32, kind="ExternalOutput")
    if dbg:
        d_qt = nc.dram_tensor("d_qt", [128, S], BF, kind="ExternalOutput")
        d_kt = nc.dram_tensor("d_kt", [128, S], BF, kind="ExternalOutput")
        d_v = nc.dram_tensor("d_v", [128, NT * 2 * (HD + 1)], BF,
                             kind="ExternalOutput")
        d_oc = nc.dram_tensor("d_oc", [128, NCORES * ROWS], BF,
                              kind="ExternalOutput")

    with tile.TileContext(nc) as tc:
        with tc.tile_pool(name="persist", bufs=1) as pp, \
             tc.tile_pool(name="dram", bufs=1, space="DRAM") as dram, \
             tc.tile_pool(name="work", bufs=4) as work, \
             tc.tile_pool(name="epool", bufs=12) as epool:

            # ---- persistent SBUF ----
            xt_sb = pp.tile([128, KC, B * S], BF, name="xt_sb")
            wq_sb = pp.tile([128, KC, W3], BF, name="wq_sb")
            wp_sb = [pp.tile([64, KC, D], BF, name=f"wp_sb{h}")
                     for h in range(HPC)]
            ba_sb = pp.tile([1, W3], BF, name="ba_sb")
            bp_sb = pp.tile([1, D], BF, name="bp_sb")
            ones_sb = pp.tile([1, 128], BF, name="ones_sb")
            c_bias = pp.tile([128, 1], F32, name="c_bias")
            c_scale = pp.tile([128, 1], F32, name="c_scale")
            ident = pp.tile([128, 128], BF, name="ident")
            tri = pp.tile([128, 128], BF, name="tri")
            # q,k working copies (normalized in place) + batched norm stats
            qk_all = pp.tile([128, NT, 2 * HPC * HD], BF, name="qk_all")
            n2_all = pp.tile([128, NT, 2 * HPC], F32, name="n2_all")
            rn_all = pp.tile([128, NT, 2 * HPC], F32, name="rn_all")
            # qT/kT per batch: head0 rows 0:64, head1 rows 64:128
            QT = [pp.tile([128, S], BF, name=f"qt{b}") for b in range(B)]
            KT = [pp.tile([128, S], BF, name=f"kt{b}") for b in range(B)]
            # v in [seq, hd] layout, per head augmented with a ones column
            v_sb = pp.tile([128, NT, 2 * (HD + 1)], BF, name="v_sb")
            ocT = [pp.tile([64, NCORES, ROWS], BF, name=f"ocT{h}")
                   for h in range(HPC)]

            # per-head A2A payload is oT: shard p = rows [64p:64p+64] = this
            # core's head-h oT columns destined for core p
            a2a_in = [dram.tile([64 * NCORES, ROWS], BF, name=f"a2a_in{h}")
                      for h in range(HPC)]
            a2a_out = [dram.tile([64 * NCORES, ROWS], BF, name=f"a2a_out{h}")
                       for h in range(HPC)]

            # ---- constants (ordered so phase B can start early) ----
            for kc in range(KC):
                nc.sync.dma_start(wq_sb[:, kc, :], wq[ts(kc, 128), :])
            nc.sync.dma_start(ba_sb[:], ba[:])
            nc.sync.dma_start(bp_sb[:], bp[:])
            for tq in range(4):
                for kc in range(KC):
                    nc.sync.dma_start(xt_sb[:, kc, ds(1024 * tq, 1024)],
                                      xt[ts(kc, 128), ds(1024 * tq, 1024)])
            for kc in range(KC):
                for h in range(HPC):
                    nc.sync.dma_start(wp_sb[h][:, kc, :],
                                      wp[128 * kc + 64 * h:
                                         128 * kc + 64 * h + 64, :])
            nc.gpsimd.memset(ones_sb[:], 1.0)
            nc.gpsimd.memset(c_bias[:], HALF_LN8)
            nc.gpsimd.memset(c_scale[:], -0.5)
            nc.gpsimd.memset(v_sb[:], 1.0)
            make_identity(nc, ident[:])
            # tri[k, q] = 1 where q >= k (valid causal), else 0
            nc.gpsimd.memset(tri[:], 1.0)
            nc.gpsimd.affine_select(
                out=tri[:], in_=tri[:], compare_op=mybir.AluOpType.is_ge,
                fill=0.0, base=0, pattern=[[1, 128]], channel_multiplier=-1)

            # ---- phase B1: qkv matmuls + norm stats ----
            ps_qkv_ctx = tc.tile_pool(name="ps_qkv", bufs=4, space="PSUM")
            ps_qkv = ps_qkv_ctx.__enter__()
            for t in range(NT):
                ps = ps_qkv.tile([128, W3], F32, tag="ps", name=f"ps{t}")
                for kc in range(KC):
                    nc.tensor.matmul(ps[:], lhsT=xt_sb[:, kc, ts(t, 128)],
                                     rhs=wq_sb[:, kc, :], start=(kc == 0),
                                     stop=(not with_bias and kc == KC - 1))
                if with_bias:
                    nc.tensor.matmul(ps[:], lhsT=ones_sb[:, 0:128],
                                     rhs=ba_sb[:], start=False, stop=True)
                nc.scalar.copy(qk_all[:, t, :], ps[:, 0:256])
                nc.vector.tensor_copy(
                    v_sb[:, t, :].rearrange(
                        "p (h e) -> p h e", e=HD + 1)[:, :, 0:HD],
                    ps[:, 256:384].rearrange("p (h e) -> p h e", e=HD))
                sq = work.tile([128, 2 * HPC * HD], BF, tag="sq",
                               name=f"sq{t}")
                nc.vector.tensor_mul(sq[:], qk_all[:, t, :], qk_all[:, t, :])
                nc.vector.reduce_sum(
                    n2_all[:, t, :], sq[:].rearrange("p (g e) -> p g e", e=HD),
                    axis=mybir.AxisListType.X)

            # batched rnorm = sqrt(8)/sqrt(n2): one Ln + one Exp (one table
            # set load each instead of per-tile thrash)
            nc.scalar.activation(rn_all[:].rearrange("p a b -> p (a b)"),
                                 n2_all[:].rearrange("p a b -> p (a b)"), AF.Ln)
            nc.scalar.activation(rn_all[:].rearrange("p a b -> p (a b)"),
                                 rn_all[:].rearrange("p a b -> p (a b)"), AF.Exp,
                                 scale=c_scale[:], bias=c_bias[:])

            ps_qkv_ctx.__exit__(None, None, None)
            ps_tr_ctx = tc.tile_pool(name="ps_tr", bufs=2, space="PSUM")
            ps_tr = ps_tr_ctx.__enter__()
            psB_ctx = tc.tile_pool(name="psB", bufs=2, space="PSUM")
            psB = psB_ctx.__enter__()
            psC_ctx = tc.tile_pool(name="psC", bufs=4, space="PSUM")
            psC = psC_ctx.__enter__()

            # ---- phase B2: normalize + transpose ----
            for t in range(NT):
                b_, tt = divmod(t, TPB)
                nc.vector.tensor_tensor(
                    qk_all[:, t, :].rearrange("p (g e) -> p g e", e=HD),
                    qk_all[:, t, :].rearrange("p (g e) -> p g e", e=HD),
                    rn_all[:, t, :, None].broadcast_to([128, 4, HD]), op=MUL)
                for src0, dst in ((0, QT[b_]), (128, KT[b_])):
                    trp = ps_tr.tile([128, 128], BF, tag="tr",
                                     name=f"tr{t}_{src0}")
                    nc.tensor.transpose(
                        trp[:], qk_all[:, t, src0:src0 + 128], ident[:])
                    nc.vector.tensor_copy(dst[:, ts(tt, 128)], trp[:])

            # ---- phase C: attention (head-major for A2A pipelining) ----
            for h in range(HPC):
                for b_ in range(B):
                    for j in range(NSPAN):
                        oaccs = [work.tile([128, HD], BF, tag="oacc", bufs=8,
                                           name=f"oacc{h}_{b_}_{j}_{c}")
                                 for c in range(4)]
                        nk = 4 * j + 4
                        avs = [psC.tile([128, HD + 1], F32, tag="av",
                                        name=f"av{b_}_{j}_{h}_{c}")
                               for c in range(4)]
                        for i in range(nk):
                            d = i - 4 * j
                            c0 = max(d, 0)
                            sps = psB.tile([128, QSPAN], F32, tag="s",
                                           name=f"s{b_}_{j}_{h}_{i}")
                            # stream only the causally-needed q columns
                            nc.tensor.matmul(
                                sps[:, 128 * c0:],
                                lhsT=KT[b_][64 * h:64 * h + 64, ts(i, 128)],
                                rhs=QT[b_][64 * h:64 * h + 64,
                                           ds(j * QSPAN + 128 * c0,
                                              QSPAN - 128 * c0)],
                                start=True, stop=True)
                            e = epool.tile([128, QSPAN], BF, tag="e",
                                           name=f"e{b_}_{j}_{h}_{i}")
                            nc.scalar.activation(e[:, 128 * c0:],
                                                 sps[:, 128 * c0:], AF.Exp)
                            if d >= 0:
                                nc.vector.tensor_tensor(
                                    e[:, 128 * d:128 * (d + 1)],
                                    e[:, 128 * d:128 * (d + 1)], tri[:],
                                    op=MUL)
                            for c in range(c0, 4):
                                nc.tensor.matmul(
                                    avs[c][:],
                                    lhsT=e[:, ts(c, 128)],
                                    rhs=v_sb[:, b_ * TPB + i,
                                             65 * h:65 * h + 65],
                                    start=(i == 0), stop=(i == 4 * j + c))
                        rd = work.tile([128, 4], F32, tag="rd",
                                       name=f"rd{b_}_{j}_{h}")
                        for c in range(4):
                            nc.vector.reciprocal(rd[:, c:c + 1],
                                                 avs[c][:, HD:HD + 1])
                            oacc = oaccs[c]
                            nc.vector.tensor_scalar_mul(
                                oacc[:], avs[c][:, 0:HD], rd[:, c:c + 1])
                            g = b_ * TPB + 4 * j + c
                            trp2 = ps_tr.tile([64, 128], BF, tag="tr",
                                              name=f"ot{h}_{g}")
                            nc.tensor.transpose(trp2[:], oacc[:], ident[:])
                            ot = work.tile([64, 128], BF, tag="ot", bufs=6,
                                           name=f"ots{h}_{g}")
                            nc.vector.tensor_copy(ot[:], trp2[:])
                            nc.sync.dma_start(
                                a2a_in[h][ts(g // 4, 64),
                                          ds((g % 4) * 128, 128)],
                                ot[:])
                # per-head A2A fires as soon as this head's o is out;
                # head 0's collective overlaps head 1's attention
                nc.gpsimd.collective_compute(
                    "AllToAll", mybir.AluOpType.bypass,
                    replica_groups=[list(range(NCORES))],
                    ins=[a2a_in[h][:].opt()], outs=[a2a_out[h][:].opt()])
                nc.sync.dma_start(
                    ocT[h][:, :, :],
                    a2a_out[h][:].rearrange("(p c) w -> c p w", c=64))

            if dbg:
                nc.sync.dma_start(d_qt[:], QT[0][:])
                nc.sync.dma_start(d_kt[:], KT[0][:])
                nc.sync.dma_start(d_v[:],
                                  v_sb[:].rearrange("p a b -> p (a b)"))
                for h in range(HPC):
                    nc.sync.dma_start(
                        d_oc[64 * h:64 * h + 64, :],
                        ocT[h][:].rearrange("p a b -> p (a b)"))

            # ---- phase D: projection, split per head so the h0 half
            # overlaps head 1's attention + A2A ----
            ysbs = {}
            for rt in range(ROWS // 128):
                for half in range(2):
                    yps = psB.tile([128, 512], F32, tag="s",
                                   name=f"y0_{rt}_{half}")
                    for p in range(NCORES):
                        nc.tensor.matmul(
                            yps[:], lhsT=ocT[0][:, p, ts(rt, 128)],
                            rhs=wp_sb[0][:, p, ds(half * 512, 512)],
                            start=(p == 0),
                            stop=(not with_bias and p == NCORES - 1))
                    if with_bias:
                        nc.tensor.matmul(yps[:], lhsT=ones_sb[:, 0:128],
                                         rhs=bp_sb[:, ds(half * 512, 512)],
                                         start=False, stop=True)
                    ysb = work.tile([128, 512], F32, tag="y", bufs=8,
                                    name=f"ysb{rt}_{half}")
                    nc.vector.tensor_copy(ysb[:], yps[:])
                    ysbs[rt, half] = ysb
            for rt in range(ROWS // 128):
                for half in range(2):
                    yps = psB.tile([128, 512], F32, tag="s",
                                   name=f"y1_{rt}_{half}")
                    for p in range(NCORES):
                        nc.tensor.matmul(
                            yps[:], lhsT=ocT[1][:, p, ts(rt, 128)],
                            rhs=wp_sb[1][:, p, ds(half * 512, 512)],
                            start=(p == 0), stop=(p == NCORES - 1))
                    ysb = ysbs[rt, half]
                    nc.vector.tensor_tensor(ysb[:], ysb[:], yps[:],
                                            op=mybir.AluOpType.add)
                    nc.sync.dma_start(
                        out[ts(rt, 128), ds(half * 512, 512)], ysb[:])

            psC_ctx.__exit__(None, None, None)
            psB_ctx.__exit__(None, None, None)
            ps_tr_ctx.__exit__(None, None, None)

    nc.compile()
    return nc


_NC = None


def _get_nc(with_bias=False):
    global _NC
    if _NC is None or _NC[1] != with_bias:
        _NC = (build(with_bias=with_bias), with_bias)
    return _NC[0]


def make_in_maps(x, w_attn, b_attn, w_proj, b_proj):
    bf = ml_dtypes.bfloat16
    xt = np.ascontiguousarray(x.reshape(B * S, D).T).astype(bf)
    wp_ = np.ascontiguousarray(w_proj).astype(bf)
    bp_ = b_proj.reshape(1, D).astype(bf)
    in_maps = []
    for c in range(NCORES):
        sl = slice(128 * c, 128 * c + 128)
        wq_ = np.ascontiguousarray(np.concatenate(
            [w_attn[:, sl], w_attn[:, 1024:2048][:, sl],
             w_attn[:, 2048:3072][:, sl]], axis=1)).astype(bf)
        ba_ = np.concatenate(
            [b_attn[sl], b_attn[1024:2048][sl],
             b_attn[2048:3072][sl]]).reshape(1, W3).astype(bf)
        in_maps.append({"xt": xt, "wq": wq_, "ba": ba_, "wp": wp_, "bp": bp_})
    return in_maps


def gather_out(results):
    out = np.empty((B, S, D), np.float32)
    for c in range(NCORES):
        out[c // 4, ROWS * (c % 4):ROWS * (c % 4 + 1), :] = results[c]["out"]
    return out


def kernel(x, w_attn, b_attn, w_proj, b_proj):
    with_bias = bool(np.any(b_attn) or np.any(b_proj))
    nc = _get_nc(with_bias=with_bias)
    in_maps = make_in_maps(np.asarray(x, np.float32), np.asarray(w_attn, np.float32),
                           np.asarray(b_attn, np.float32),
                           np.asarray(w_proj, np.float32),
                           np.asarray(b_proj, np.float32))
    res = run_bass_kernel_spmd(nc, in_maps, core_ids=list(range(NCORES)))
    return gather_out(res.results)
